# revision 5
# baseline (speedup 1.0000x reference)
"""CSWin attention Bass/Trainium2 kernel (SPMD over 8 NeuronCores).

Problem: nn_CSWinAttention. B=2, H=W=56, N=2 candidates, C=128 channels,
8 heads x d=16, vertical-stripe windows Hsp=56, Wsp=7 -> 16 windows of
L=784 tokens. Plus LePE-style depthwise-3x3 rpe on the value.

Sharding: each core owns 2 windows (core c -> batch c//4, window cols
[14*(c%4), 14*(c%4)+14)). Full attention + rpe computed on-device; host
only slices/pads inputs and concatenates outputs.

v2: bf16 datapath for the attention matmuls (QK^T, AV, transposes run at
1 cyc/col on the PE instead of fp32's 4), single-instruction exp over
[112, 784] S^T tiles spanning two PSUM banks (halves ACT instruction
count), batched one-DMA-per-tensor loads, and bf16 HBM for q/k/v.
PSUM budget: st 2x2 banks + av 1x2 banks + tr 2x1 banks = 8.
"""

import numpy as np

B, Hh, Ww, Nc, Cc = 2, 56, 56, 2, 128
HEADS, Dh, WSP = 8, 16, 7
L = Hh * WSP * Nc          # 784 tokens per window
PCH = 112                  # token chunk (partition) size; 7 chunks
QC = L // PCH              # 7
H0, H1 = 512, 272          # bank-aligned split of L (512*4B = 1 psum bank)
SCALE = float(Dh) ** -0.5

_cache = {}


def _build_program():
    import concourse.bacc as bacc
    import concourse.tile as tile
    from concourse import mybir

    f32 = mybir.dt.float32
    bf16 = mybir.dt.bfloat16
    AT = mybir.AluOpType
    AF = mybir.ActivationFunctionType

    nc = bacc.Bacc("TRN2", target_bir_lowering=False, debug=False, num_devices=8)

    q_d = nc.dram_tensor("q", [Hh, 2 * WSP, Nc, Cc], bf16, kind="ExternalInput")
    k_d = nc.dram_tensor("k", [Hh, 2 * WSP, Nc, Cc], bf16, kind="ExternalInput")
    v16_d = nc.dram_tensor("v16", [Hh, 2 * WSP, Nc, Cc], bf16, kind="ExternalInput")
    v_d = nc.dram_tensor("v", [Hh, 2 * WSP + 2, Nc, Cc], f32, kind="ExternalInput")
    tapw_d = nc.dram_tensor("tapw", [Cc, 9], f32, kind="ExternalInput")
    cneg_d = nc.dram_tensor("cneg", [Cc, 1], f32, kind="ExternalInput")
    cpos_d = nc.dram_tensor("cpos", [Cc, 1], f32, kind="ExternalInput")
    mask_d = nc.dram_tensor("maskblk", [PCH, PCH], bf16, kind="ExternalInput")
    iden32_d = nc.dram_tensor("iden32", [Cc, Cc], f32, kind="ExternalInput")
    iden16_d = nc.dram_tensor("iden16", [Cc, Cc], bf16, kind="ExternalInput")
    ebc_d = nc.dram_tensor("ebc", [Cc, Cc], bf16, kind="ExternalInput")
    out_d = nc.dram_tensor("out", [Hh, 2 * WSP, Nc, Cc], f32, kind="ExternalOutput")

    with tile.TileContext(nc) as tc:
        with (
            tc.tile_pool(name="consts", bufs=1) as consts,
            tc.tile_pool(name="io", bufs=2) as io,
            tc.tile_pool(name="tr", bufs=2) as trp,
            tc.tile_pool(name="rpe", bufs=2) as rpep,
            tc.tile_pool(name="pt", bufs=12) as ptp,
            tc.tile_pool(name="post", bufs=2) as postp,
            tc.tile_pool(name="ps_st", bufs=2, space="PSUM") as ps_st,
            tc.tile_pool(name="ps_av", bufs=1, space="PSUM") as ps_av,
            tc.tile_pool(name="ps_tr", bufs=2, space="PSUM") as ps_tr,
        ):
            iden32 = consts.tile([Cc, Cc], f32)
            nc.sync.dma_start(out=iden32[:], in_=iden32_d[:])
            iden16 = consts.tile([Cc, Cc], bf16)
            nc.sync.dma_start(out=iden16[:], in_=iden16_d[:])
            maskblk = consts.tile([PCH, PCH], bf16)
            nc.sync.dma_start(out=maskblk[:], in_=mask_d[:])
            tapw = consts.tile([Cc, 9], f32)
            nc.sync.dma_start(out=tapw[:], in_=tapw_d[:])
            cneg = consts.tile([Cc, 1], f32)
            nc.sync.dma_start(out=cneg[:], in_=cneg_d[:])
            cpos = consts.tile([Cc, 1], f32)
            nc.sync.dma_start(out=cpos[:], in_=cpos_d[:])
            ebc = consts.tile([Cc, Cc], bf16)
            nc.sync.dma_start(out=ebc[:], in_=ebc_d[:])

            win = [dict() for _ in range(2)]
            for jj in range(2):  # loads + transposes + rpe, both windows
                x0 = WSP * jj

                # ---------- loads ----------
                q_sb = io.tile([PCH, QC, Cc], bf16, tag="q_sb")
                k_sb = io.tile([PCH, QC, Cc], bf16, tag="k_sb")
                for c in range(QC):
                    nc.sync.dma_start(
                        out=q_sb[:, c, :],
                        in_=q_d[8 * c:8 * c + 8, x0:x0 + WSP, :, :].rearrange(
                            "y x n c -> y x (n c)"
                        ),
                    )
                    nc.sync.dma_start(
                        out=k_sb[:, c, :],
                        in_=k_d[8 * c:8 * c + 8, x0:x0 + WSP, :, :].rearrange(
                            "y x n c -> y x (n c)"
                        ),
                    )
                # V with per-head [16 cols | ones | pad] 24-blocks for AV lhsT
                v_aug = io.tile([PCH, QC, HEADS, 24], bf16, tag="v_aug")
                for c in range(QC):
                    nc.sync.dma_start(
                        out=v_aug[:, c, :, 0:Dh],
                        in_=v16_d[8 * c:8 * c + 8, x0:x0 + WSP, :, :]
                        .rearrange("y x n (h d) -> y x (n h) d", h=HEADS),
                    )
                nc.vector.memset(v_aug[:, :, :, Dh:Dh + 1], 1.0)
                # V with x halo for the conv (126 = 7y * 9x * 2n)
                v_ext = io.tile([126, 8, Cc], f32, tag="v_ext")
                for b8 in range(8):
                    nc.sync.dma_start(
                        out=v_ext[:, b8, :],
                        in_=v_d[7 * b8:7 * b8 + 7, x0:x0 + WSP + 2, :, :]
                        .rearrange("y x n c -> y x (n c)"),
                    )

                # ---------- transposes (bf16 for q/k) ----------
                qt_ev = trp.tile([Cc, QC, PCH], bf16, tag="qt_ev")
                qt_od = trp.tile([PCH, QC, PCH], bf16, tag="qt_od")
                kt_ev = trp.tile([Cc, QC, PCH], bf16, tag="kt_ev")
                kt_od = trp.tile([PCH, QC, PCH], bf16, tag="kt_od")
                for c in range(QC):
                    for src, dst_ev, dst_od in ((q_sb, qt_ev, qt_od),
                                                (k_sb, kt_ev, kt_od)):
                        t1 = ps_tr.tile([Cc, PCH], bf16, tag="tr")
                        nc.tensor.transpose(
                            t1[:], src[:, c, :], iden16[0:PCH, 0:PCH]
                        )
                        nc.vector.tensor_copy(dst_ev[:, c, :], t1[:])
                        t2 = ps_tr.tile([PCH, PCH], bf16, tag="tr")
                        nc.tensor.transpose(
                            t2[:], src[:, c, Dh:Cc], iden16[0:PCH, 0:PCH]
                        )
                        nc.vector.tensor_copy(dst_od[:, c, :], t2[:])

                vt_ext = trp.tile([Cc, 8, 7, 9, 2], f32, tag="vt_ext")
                for b8 in range(8):
                    t3 = ps_tr.tile([Cc, 126], f32, tag="tr")
                    nc.tensor.transpose(
                        t3[:], v_ext[:, b8, :], iden32[0:126, 0:126]
                    )
                    nc.vector.tensor_copy(
                        vt_ext[:, b8, :, :, :].rearrange("c y x n -> c (y x n)"),
                        t3[:],
                    )

                # ---------- rpe (GPSIMD + DVE) ----------
                vs_pad = rpep.tile([Cc, 58, 9], f32, tag="vs_pad")
                nc.gpsimd.memset(vs_pad[:], 0.0)
                nc.gpsimd.tensor_tensor(
                    vs_pad[:, 1:57, :].rearrange("c (yb y) x -> c yb y x", y=7),
                    vt_ext[:, :, :, :, 0],
                    vt_ext[:, :, :, :, 1],
                    AT.add,
                )
                conv_a = rpep.tile([Cc, 56, 7], f32, tag="conv_a")
                conv_b = rpep.tile([Cc, 56, 7], f32, tag="conv_b")
                acc_src = None
                for t in range(9):
                    ky, kx = t // 3, t % 3
                    shifted = vs_pad[:, ky:ky + 56, kx:kx + 7]
                    dst = conv_a if t % 2 == 0 else conv_b
                    if t == 0:
                        nc.vector.tensor_scalar(
                            dst[:], shifted, tapw[:, t:t + 1], None, AT.mult
                        )
                    else:
                        nc.vector.scalar_tensor_tensor(
                            dst[:], shifted, tapw[:, t:t + 1], acc_src[:],
                            AT.mult, AT.add,
                        )
                    acc_src = dst
                # cvs = conv - center*vs   (on interior x: vs_pad x 1..8)
                cvs = rpep.tile([Cc, 56, 7], f32, tag="cvs")
                nc.vector.scalar_tensor_tensor(
                    cvs[:], vs_pad[:, 1:57, 1:8], cneg[:], acc_src[:],
                    AT.mult, AT.add,
                )
                # rpe[c, y, x, n] = center*v + cvs
                rpe = rpep.tile([Cc, 56, 7, 2], f32, tag="rpe")
                for n in range(2):
                    nc.vector.scalar_tensor_tensor(
                        rpe[:, :, :, n],
                        vt_ext[:, :, :, 1:8, n].rearrange("c yb y x -> c (yb y) x"),
                        cpos[:],
                        cvs[:],
                        AT.mult, AT.add,
                    )

                win[jj].update(q_sb=q_sb, k_sb=k_sb, v_aug=v_aug,
                               qt_ev=qt_ev, qt_od=qt_od, kt_ev=kt_ev,
                               kt_od=kt_od, rpe=rpe)

            for jj in range(2):  # attention + final, both windows
                x0 = WSP * jj
                v_aug = win[jj]["v_aug"]
                qt_ev = win[jj]["qt_ev"]; qt_od = win[jj]["qt_od"]
                kt_ev = win[jj]["kt_ev"]; kt_od = win[jj]["kt_od"]
                rpe = win[jj]["rpe"]

                # ---------- attention ----------
                scaled = {}
                for set_i in range(2):
                    heads = [4 * set_i + i for i in range(4)]
                    av = ps_av.tile([Cc, L], f32, tag="av")  # 2 banks
                    pairs = [
                        (kt_ev, qt_ev, (heads[0], heads[2])),
                        (kt_od, qt_od, (heads[1], heads[3])),
                    ]

                    def emit_av(qc, pt_by_head):
                        # 4 col-tiled matmuls back-to-back per half so the
                        # 32-col sub-arrays run them concurrently
                        for c0, c1 in ((0, H0), (H0, L)):
                            for h in heads:
                                j = h - 4 * set_i
                                pt = pt_by_head[h]
                                nc.tensor.matmul(
                                    av[32 * j:32 * j + Dh + 1, c0:c1],
                                    v_aug[:, qc, h, 0:Dh + 1],
                                    pt[:, c0:c1],
                                    start=(qc == 0), stop=(qc == QC - 1),
                                    tile_position=(0, 32 * j),
                                    skip_group_check=True,
                                )

                    prev_pt = None
                    for qc in range(QC):
                        # AV for the previous chunk goes first: its inputs
                        # are long since ready, so the 4-head col-tiled
                        # quads issue back-to-back and overlap on the
                        # 32-col sub-arrays.
                        if prev_pt is not None:
                            emit_av(qc - 1, prev_pt)
                        pt_by_head = {}
                        for kt, qt, phs in pairs:
                            bases = (64 * set_i, 64 * set_i + 32)
                            for h, base in zip(phs, bases):
                                st = ps_st.tile([PCH, L], f32, tag="st")
                                qtf = qt[base:base + Dh, :, :].rearrange(
                                    "k a b -> k (a b)"
                                )
                                nc.tensor.matmul(
                                    st[:, 0:H0],
                                    kt[base:base + Dh, qc, :],
                                    qtf[:, 0:H0],
                                    start=True, stop=True,
                                    tile_position=(base, 0),
                                )
                                nc.tensor.matmul(
                                    st[:, H0:L],
                                    kt[base:base + Dh, qc, :],
                                    qtf[:, H0:L],
                                    start=True, stop=True,
                                    tile_position=(base, 0),
                                )
                                pt = ptp.tile([PCH, L], bf16, tag="pt")
                                nc.scalar.activation(
                                    pt[:], st[:], AF.Exp, scale=SCALE
                                )
                                # pair mask: zero the partner entries in the
                                # diagonal 112-block, multiplicatively, off
                                # the PE<->ACT critical path (idle GPSIMD)
                                blk = pt[:, qc * PCH:(qc + 1) * PCH]
                                nc.gpsimd.tensor_tensor(
                                    blk, blk, maskblk[:], AT.mult
                                )
                                pt_by_head[h] = pt
                        prev_pt = pt_by_head
                    emit_av(QC - 1, prev_pt)

                    # ----- normalize: divisor broadcast + reciprocal -----
                    av_sb = postp.tile([Cc, L], bf16, tag="av_sb")
                    nc.vector.tensor_copy(av_sb[:], av[:])
                    d = ps_av.tile([Cc, L], f32, tag="av")
                    nc.tensor.matmul(d[:, 0:H0], ebc[:], av_sb[:, 0:H0],
                                     start=True, stop=True)
                    nc.tensor.matmul(d[:, H0:L], ebc[:], av_sb[:, H0:L],
                                     start=True, stop=True)
                    drec = postp.tile([Cc, L], f32, tag="drec")
                    nc.vector.reciprocal_approx_fast(drec[:], d[:])
                    sc = postp.tile([Cc, L], bf16, tag="scaled")
                    nc.vector.tensor_tensor(sc[:], av_sb[:], drec[:], AT.mult)
                    scaled[set_i] = sc

                # ---------- final: transpose back, add rpe, store ----------
                final_sb = postp.tile([PCH, QC, Cc], f32, tag="final")
                rpe_flat = rpe[:].rearrange("c y x n -> c (y x n)")
                for qc in range(QC):
                    trr = ps_tr.tile([PCH, Cc], f32, tag="tr")
                    nc.tensor.transpose(
                        trr[:], rpe_flat[:, qc * PCH:(qc + 1) * PCH], iden32[:]
                    )
                    rpe_tb = postp.tile([PCH, Cc], f32, tag="rpe_tb")
                    nc.vector.tensor_copy(rpe_tb[:], trr[:])
                    for set_i in range(2):
                        ta = ps_tr.tile([PCH, Cc], bf16, tag="tr")
                        nc.tensor.transpose(
                            ta[:],
                            scaled[set_i][:, qc * PCH:(qc + 1) * PCH],
                            iden16[:],
                        )
                        nc.vector.tensor_tensor(
                            final_sb[:, qc, :].rearrange(
                                "p (g k) -> p g k", g=HEADS
                            )[:, 4 * set_i:4 * set_i + 4, :],
                            ta[:].rearrange("p (g k) -> p g k", g=4)[:, :, 0:Dh],
                            rpe_tb[:].rearrange("p (g k) -> p g k", g=HEADS)[
                                :, 4 * set_i:4 * set_i + 4, :
                            ],
                            AT.add,
                        )
                for c in range(QC):
                    nc.sync.dma_start(
                        out=out_d[8 * c:8 * c + 8, x0:x0 + WSP, :, :].rearrange(
                            "y x n c -> y x (n c)"
                        ),
                        in_=final_sb[:, c, :],
                    )

    nc.compile()
    return nc


def _host_inputs(query, key, value, conv_w):
    """Build the 8 per-core input dicts."""
    from ml_dtypes import bfloat16

    query = np.ascontiguousarray(query, dtype=np.float32)
    key = np.ascontiguousarray(key, dtype=np.float32)
    value = np.ascontiguousarray(value, dtype=np.float32)
    conv_w = np.asarray(conv_w, dtype=np.float32)

    q16 = query.astype(bfloat16)
    k16 = key.astype(bfloat16)
    v16 = value.astype(bfloat16)

    tapw = conv_w[:, 0].reshape(Cc, 9).copy()
    center = conv_w[:, 0, 1, 1].reshape(Cc, 1).copy()
    cneg = np.ascontiguousarray(-center)
    maskblk = np.ones((PCH, PCH), np.float32)
    idx = np.arange(PCH)
    maskblk[idx, idx ^ 1] = 0.0
    iden = np.eye(Cc, dtype=np.float32)
    ebc = np.zeros((Cc, Cc), np.float32)
    for j in range(4):
        ebc[32 * j + Dh, 32 * j:32 * j + Dh] = 1.0

    in_maps = []
    for c in range(8):
        b, jblk = c // 4, c % 4
        xs = 14 * jblk
        v_sl = np.zeros((Hh, 16, Nc, Cc), np.float32)
        v_sl[:, 1:15] = value[b, :, xs:xs + 14]
        if xs - 1 >= 0:
            v_sl[:, 0] = value[b, :, xs - 1]
        if xs + 14 < Ww:
            v_sl[:, 15] = value[b, :, xs + 14]
        in_maps.append({
            "q": np.ascontiguousarray(q16[b, :, xs:xs + 14]),
            "k": np.ascontiguousarray(k16[b, :, xs:xs + 14]),
            "v16": np.ascontiguousarray(v16[b, :, xs:xs + 14]),
            "v": v_sl,
            "tapw": tapw,
            "cneg": cneg,
            "cpos": center,
            "maskblk": maskblk.astype(bfloat16),
            "iden32": iden,
            "iden16": iden.astype(bfloat16),
            "ebc": ebc.astype(bfloat16),
        })
    return in_maps


def _run(in_maps, trace=False):
    from concourse.bass_utils import run_bass_kernel_spmd

    if "nc" not in _cache:
        _cache["nc"] = _build_program()
    return run_bass_kernel_spmd(
        _cache["nc"], in_maps, core_ids=list(range(8)), trace=trace
    )


def kernel(query, key, value, conv_w):
    in_maps = _host_inputs(query, key, value, conv_w)
    res = _run(in_maps)
    out = np.zeros((B, Hh, Ww, Nc, Cc), np.float32)
    for c in range(8):
        b, jblk = c // 4, c % 4
        out[b, :, 14 * jblk:14 * jblk + 14] = res.results[c]["out"]
    return out


# revision 13
# speedup vs baseline: 1.1909x; 1.1909x over previous
"""CSWin attention Bass/Trainium2 kernel (SPMD over 8 NeuronCores).

Problem: nn_CSWinAttention. B=2, H=W=56, N=2 candidates, C=128 channels,
8 heads x d=16, vertical-stripe windows Hsp=56, Wsp=7 -> 16 windows of
L=784 tokens. Plus LePE-style depthwise-3x3 rpe on the value.

Sharding: each core owns 2 windows (core c -> batch c//4, window cols
[14*(c%4), 14*(c%4)+14)). Full attention + rpe computed on-device; host
only slices/pads inputs and concatenates outputs.

v3: bf16 datapath; all transposes moved off the PE onto the DMA xbar
(InstDmaTransposeAnt, 16x128 tiles, blocked 3D-out form); softmax
normalization folded into the transposed-back layout (per-partition
scalar broadcast) replacing the ebc broadcast matmul; one exp per
(head, chunk) over a [112, 784] PSUM tile spanning 2 banks.
PSUM budget: st 3x2 banks + av 1x2 banks = 8.
"""

import numpy as np

B, Hh, Ww, Nc, Cc = 2, 56, 56, 2, 128
HEADS, Dh, WSP = 8, 16, 7
L = Hh * WSP * Nc          # 784 tokens per window
PCH = 112                  # token chunk (partition) size; 7 chunks
QC = L // PCH              # 7
H0 = 512                   # bank-aligned split of L (512*4B = 1 psum bank)
SCALE = float(Dh) ** -0.5

_cache = {}


def _build_program():
    import concourse.bacc as bacc
    import concourse.tile as tile
    from concourse import mybir

    f32 = mybir.dt.float32
    bf16 = mybir.dt.bfloat16
    AT = mybir.AluOpType
    AF = mybir.ActivationFunctionType

    nc = bacc.Bacc("TRN2", target_bir_lowering=False, debug=False, num_devices=8)

    q_d = nc.dram_tensor("q", [Hh, 2 * WSP, Nc, Cc], bf16, kind="ExternalInput")
    k_d = nc.dram_tensor("k", [Hh, 2 * WSP, Nc, Cc], bf16, kind="ExternalInput")
    v_d = nc.dram_tensor("v", [Hh, 2 * WSP + 2, Nc, Cc], bf16, kind="ExternalInput")
    tapw_d = nc.dram_tensor("tapw", [Cc, 9], f32, kind="ExternalInput")
    cneg_d = nc.dram_tensor("cneg", [Cc, 1], f32, kind="ExternalInput")
    cpos_d = nc.dram_tensor("cpos", [Cc, 1], f32, kind="ExternalInput")
    mask_d = nc.dram_tensor("maskblk", [PCH, PCH], bf16, kind="ExternalInput")
    out_d = nc.dram_tensor("out", [Hh, 2 * WSP, Nc, Cc], f32, kind="ExternalOutput")

    with tile.TileContext(nc) as tc:
        with (
            tc.tile_pool(name="consts", bufs=1) as consts,
            tc.tile_pool(name="io", bufs=2) as io,
            tc.tile_pool(name="tr", bufs=2) as trp,
            tc.tile_pool(name="rpe", bufs=2) as rpep,
            tc.tile_pool(name="pt", bufs=12) as ptp,
            tc.tile_pool(name="post", bufs=2) as postp,
            tc.tile_pool(name="ps_st", bufs=3, space="PSUM") as ps_st,
            tc.tile_pool(name="ps_av", bufs=1, space="PSUM") as ps_av,
        ):
            maskblk = consts.tile([PCH, PCH], bf16)
            nc.sync.dma_start(out=maskblk[:], in_=mask_d[:])
            tapw = consts.tile([Cc, 9], f32)
            nc.sync.dma_start(out=tapw[:], in_=tapw_d[:])
            cneg = consts.tile([Cc, 1], f32)
            nc.sync.dma_start(out=cneg[:], in_=cneg_d[:])
            cpos = consts.tile([Cc, 1], f32)
            nc.sync.dma_start(out=cpos[:], in_=cpos_d[:])

            win = [dict() for _ in range(2)]
            for jj in range(2):  # loads + transposes + rpe, both windows
                x0 = WSP * jj

                # ---------- loads ----------
                q_sb = io.tile([PCH, QC, Cc], bf16, tag="q_sb")
                k_sb = io.tile([PCH, QC, Cc], bf16, tag="k_sb")
                for c in range(QC):
                    nc.sync.dma_start(
                        out=q_sb[:, c, :],
                        in_=q_d[8 * c:8 * c + 8, x0:x0 + WSP, :, :].rearrange(
                            "y x n c -> y x (n c)"
                        ),
                    )
                    nc.sync.dma_start(
                        out=k_sb[:, c, :],
                        in_=k_d[8 * c:8 * c + 8, x0:x0 + WSP, :, :].rearrange(
                            "y x n c -> y x (n c)"
                        ),
                    )
                # V with per-head [16 cols | ones | pad] 24-blocks for AV lhsT
                v_aug = io.tile([PCH, QC, HEADS, 24], bf16, tag="v_aug")
                for c in range(QC):
                    nc.sync.dma_start(
                        out=v_aug[:, c, :, 0:Dh],
                        in_=v_d[8 * c:8 * c + 8, 1 + x0:1 + x0 + WSP, :, :]
                        .rearrange("y x n (h d) -> y x (n h) d", h=HEADS),
                    )
                nc.vector.memset(v_aug[:, :, :, Dh:Dh + 1], 1.0)
                # V with x halo for the conv (126 = 7y * 9x * 2n; 2 pad rows)
                v_ext = io.tile([Cc, 8, Cc], bf16, tag="v_ext")
                nc.vector.memset(v_ext[96:128, :, :], 0.0)
                for b8 in range(8):
                    nc.sync.dma_start(
                        out=v_ext[0:126, b8, :],
                        in_=v_d[7 * b8:7 * b8 + 7, x0:x0 + WSP + 2, :, :]
                        .rearrange("y x n c -> y x (n c)"),
                    )

                # ---------- shifted copies for the odd-head layout ----------
                # odd head h channels live at 16h; shifting by 16 puts them at
                # 32-aligned partition bases after the transpose.
                q_sh = io.tile([PCH, QC, Cc], bf16, tag="q_sh")
                nc.vector.tensor_copy(q_sh[:, :, 0:PCH], q_sb[:, :, Dh:Cc])
                k_sh = io.tile([PCH, QC, Cc], bf16, tag="k_sh")
                nc.vector.tensor_copy(k_sh[:, :, 0:PCH], k_sb[:, :, Dh:Cc])

                # ---------- transposes on the DMA xbar ----------
                # out[p, c, t] = in[t, 128c + p] -> per-chunk [112,128]^T
                qt_ev = trp.tile([Cc, QC, PCH], bf16, tag="qt_ev")
                nc.sync.dma_start(out=qt_ev[:], in_=q_sb[:], transpose=True)
                qt_od = trp.tile([Cc, QC, PCH], bf16, tag="qt_od")
                nc.sync.dma_start(out=qt_od[:], in_=q_sh[:], transpose=True)
                kt_ev = trp.tile([Cc, QC, PCH], bf16, tag="kt_ev")
                nc.sync.dma_start(out=kt_ev[:], in_=k_sb[:], transpose=True)
                kt_od = trp.tile([Cc, QC, PCH], bf16, tag="kt_od")
                nc.sync.dma_start(out=kt_od[:], in_=k_sh[:], transpose=True)
                # v transpose for the conv: [128(126 tok+pad), 8, 128] ->
                # vt8[ch, b, tok]
                vt8 = trp.tile([Cc, 8, Cc], bf16, tag="vt8")
                nc.sync.dma_start(out=vt8[:], in_=v_ext[:], transpose=True)
                vt5 = vt8[:, :, 0:126].rearrange(
                    "c b (y x n) -> c b y x n", y=7, x=9
                )

                # ---------- rpe (GPSIMD + DVE) ----------
                vs_pad = rpep.tile([Cc, 58, 9], bf16, tag="vs_pad")
                nc.gpsimd.memset(vs_pad[:], 0.0)
                nc.gpsimd.tensor_tensor(
                    vs_pad[:, 1:57, :].rearrange("c (yb y) x -> c yb y x", y=7),
                    vt5[:, :, :, :, 0],
                    vt5[:, :, :, :, 1],
                    AT.add,
                )
                conv_a = rpep.tile([Cc, 56, 7], f32, tag="conv_a")
                conv_b = rpep.tile([Cc, 56, 7], f32, tag="conv_b")
                acc_src = None
                for t in range(9):
                    ky, kx = t // 3, t % 3
                    shifted = vs_pad[:, ky:ky + 56, kx:kx + 7]
                    dst = conv_a if t % 2 == 0 else conv_b
                    if t == 0:
                        nc.vector.tensor_scalar(
                            dst[:], shifted, tapw[:, t:t + 1], None, AT.mult
                        )
                    else:
                        nc.vector.scalar_tensor_tensor(
                            dst[:], shifted, tapw[:, t:t + 1], acc_src[:],
                            AT.mult, AT.add,
                        )
                    acc_src = dst
                # cvs = conv - center*vs   (on interior x: vs_pad x 1..8)
                cvs = rpep.tile([Cc, 56, 7], f32, tag="cvs")
                nc.vector.scalar_tensor_tensor(
                    cvs[:], vs_pad[:, 1:57, 1:8], cneg[:], acc_src[:],
                    AT.mult, AT.add,
                )
                # rpe[c, y, x, n] = center*v + cvs
                rpe = rpep.tile([Cc, 56, 7, 2], bf16, tag="rpe")
                for n in range(2):
                    for yb in range(8):
                        nc.vector.scalar_tensor_tensor(
                            rpe[:, 7 * yb:7 * yb + 7, :, n],
                            vt5[:, yb, :, 1:8, n],
                            cpos[:],
                            cvs[:, 7 * yb:7 * yb + 7, :],
                            AT.mult, AT.add,
                        )
                # repack into 128-padded chunks so the transpose-back runs
                # on the DMA xbar too
                rpe2 = rpep.tile([Cc, QC, Cc], bf16, tag="rpe2")
                nc.vector.tensor_copy(
                    rpe2[:, :, 0:PCH],
                    rpe[:].rearrange("c (q y) x n -> c q (y x n)", y=8),
                )

                win[jj].update(v_aug=v_aug, qt_ev=qt_ev, qt_od=qt_od,
                               kt_ev=kt_ev, kt_od=kt_od, rpe2=rpe2)

            for jj in range(2):  # attention + final, both windows
                x0 = WSP * jj
                v_aug = win[jj]["v_aug"]
                qt_ev = win[jj]["qt_ev"]; qt_od = win[jj]["qt_od"]
                kt_ev = win[jj]["kt_ev"]; kt_od = win[jj]["kt_od"]
                rpe2 = win[jj]["rpe2"]

                # rpe transposed back: [128(112 tok+pad), qc, ch]
                rpe_tb = postp.tile([Cc, QC, Cc], bf16, tag="rpe_tb")
                nc.sync.dma_start(out=rpe_tb[:], in_=rpe2[:], transpose=True)

                final_sb = postp.tile([PCH, QC, Cc], f32, tag="final")

                for set_i in range(2):
                    heads = [4 * set_i + i for i in range(4)]
                    av = ps_av.tile([Cc, L], f32, tag="av")  # 2 banks
                    pairs = [
                        (kt_ev, qt_ev, (heads[0], heads[2])),
                        (kt_od, qt_od, (heads[1], heads[3])),
                    ]

                    def emit_av(qc, pt_by_head):
                        # 4 col-tiled matmuls back-to-back per half so the
                        # 32-col sub-arrays run them concurrently
                        for c0, c1 in ((0, H0), (H0, L)):
                            for h in heads:
                                j = h - 4 * set_i
                                pt = pt_by_head[h]
                                nc.tensor.matmul(
                                    av[32 * j:32 * j + Dh + 1, c0:c1],
                                    v_aug[:, qc, h, 0:Dh + 1],
                                    pt[:, c0:c1],
                                    start=(qc == 0), stop=(qc == QC - 1),
                                    tile_position=(0, 32 * j),
                                    skip_group_check=True,
                                )

                    prev_pt = None
                    for qc in range(QC):
                        # AV for the previous chunk goes first: its inputs
                        # are long since ready, so the 4-head col-tiled
                        # quads issue back-to-back and overlap on the
                        # 32-col sub-arrays.
                        if prev_pt is not None:
                            emit_av(qc - 1, prev_pt)
                        pt_by_head = {}
                        for kt, qt, phs in pairs:
                            bases = (64 * set_i, 64 * set_i + 32)
                            for h, base in zip(phs, bases):
                                st = ps_st.tile([PCH, L], f32, tag="st")
                                qtf = qt[base:base + Dh, :, :].rearrange(
                                    "k a b -> k (a b)"
                                )
                                nc.tensor.matmul(
                                    st[:, 0:H0],
                                    kt[base:base + Dh, qc, :],
                                    qtf[:, 0:H0],
                                    start=True, stop=True,
                                    tile_position=(base, 0),
                                )
                                nc.tensor.matmul(
                                    st[:, H0:L],
                                    kt[base:base + Dh, qc, :],
                                    qtf[:, H0:L],
                                    start=True, stop=True,
                                    tile_position=(base, 0),
                                )
                                pt = ptp.tile([PCH, L], bf16, tag="pt")
                                nc.scalar.activation(
                                    pt[:], st[:], AF.Exp, scale=SCALE
                                )
                                # pair mask: zero the partner entries in the
                                # diagonal 112-block, multiplicatively, off
                                # the PE<->ACT critical path (idle GPSIMD)
                                blk = pt[:, qc * PCH:(qc + 1) * PCH]
                                nc.gpsimd.tensor_tensor(
                                    blk, blk, maskblk[:], AT.mult
                                )
                                pt_by_head[h] = pt
                        prev_pt = pt_by_head
                    emit_av(QC - 1, prev_pt)

                    # ----- normalize in the transposed-back layout -----
                    # av rows: head j data at 32j..32j+16, denom at 32j+16.
                    av_sb = postp.tile([Cc, QC, Cc], bf16, tag="av_sb")
                    nc.vector.tensor_copy(
                        av_sb[:, :, 0:PCH],
                        av[:].rearrange("c (q t) -> c q t", t=PCH),
                    )
                    ta = postp.tile([Cc, QC, Cc], bf16, tag="ta")
                    nc.sync.dma_start(out=ta[:], in_=av_sb[:], transpose=True)
                    # denominators -> fp32 -> reciprocal (tiny: 28 cols)
                    dcp = postp.tile([PCH, QC, 4], f32, tag="dcp")
                    nc.vector.tensor_copy(
                        dcp[:],
                        ta[0:PCH].rearrange("p q (j r) -> p q j r", r=32)[
                            :, :, :, Dh
                        ],
                    )
                    rec = postp.tile([PCH, QC, 4], f32, tag="rec")
                    nc.vector.reciprocal_approx_fast(rec[:], dcp[:])
                    # out = ta * (1/denom) + rpe, per (chunk, head)
                    for qc in range(QC):
                        for j in range(4):
                            h = 4 * set_i + j
                            nc.vector.scalar_tensor_tensor(
                                final_sb[:, qc, Dh * h:Dh * h + Dh],
                                ta[0:PCH, qc, 32 * j:32 * j + Dh],
                                rec[:, qc, j:j + 1],
                                rpe_tb[0:PCH, qc, Dh * h:Dh * h + Dh],
                                AT.mult, AT.add,
                            )

                for c in range(QC):
                    nc.sync.dma_start(
                        out=out_d[8 * c:8 * c + 8, x0:x0 + WSP, :, :].rearrange(
                            "y x n c -> y x (n c)"
                        ),
                        in_=final_sb[:, c, :],
                    )

    nc.compile()
    return nc


def _host_inputs(query, key, value, conv_w):
    """Build the 8 per-core input dicts."""
    from ml_dtypes import bfloat16

    query = np.ascontiguousarray(query, dtype=np.float32)
    key = np.ascontiguousarray(key, dtype=np.float32)
    value = np.ascontiguousarray(value, dtype=np.float32)
    conv_w = np.asarray(conv_w, dtype=np.float32)

    q16 = query.astype(bfloat16)
    k16 = key.astype(bfloat16)
    v16 = value.astype(bfloat16)

    tapw = conv_w[:, 0].reshape(Cc, 9).copy()
    center = conv_w[:, 0, 1, 1].reshape(Cc, 1).copy()
    cneg = np.ascontiguousarray(-center)
    maskblk = np.ones((PCH, PCH), np.float32)
    idx = np.arange(PCH)
    maskblk[idx, idx ^ 1] = 0.0

    in_maps = []
    for c in range(8):
        b, jblk = c // 4, c % 4
        xs = 14 * jblk
        v_sl = np.zeros((Hh, 16, Nc, Cc), bfloat16)
        v_sl[:, 1:15] = v16[b, :, xs:xs + 14]
        if xs - 1 >= 0:
            v_sl[:, 0] = v16[b, :, xs - 1]
        if xs + 14 < Ww:
            v_sl[:, 15] = v16[b, :, xs + 14]
        in_maps.append({
            "q": np.ascontiguousarray(q16[b, :, xs:xs + 14]),
            "k": np.ascontiguousarray(k16[b, :, xs:xs + 14]),
            "v": v_sl,
            "tapw": tapw,
            "cneg": cneg,
            "cpos": center,
            "maskblk": maskblk.astype(bfloat16),
        })
    return in_maps


def _run(in_maps, trace=False):
    from concourse.bass_utils import run_bass_kernel_spmd

    if "nc" not in _cache:
        _cache["nc"] = _build_program()
    return run_bass_kernel_spmd(
        _cache["nc"], in_maps, core_ids=list(range(8)), trace=trace
    )


def kernel(query, key, value, conv_w):
    in_maps = _host_inputs(query, key, value, conv_w)
    res = _run(in_maps)
    out = np.zeros((B, Hh, Ww, Nc, Cc), np.float32)
    for c in range(8):
        b, jblk = c // 4, c % 4
        out[b, :, 14 * jblk:14 * jblk + 14] = res.results[c]["out"]
    return out
